# revision 8
# baseline (speedup 1.0000x reference)
import numpy as np

B, J, DIM, H = 131072, 17, 2, 32
N_VIS, N_MASK = 12, 5
NCORES = 8
BS = B // NCORES  # rows per core
P = 128           # rows per tile (partitions)
NT = BS // P      # tiles per core
QSCALE = 126.5    # int8 quantization range


def _build_consts(positions, up_W, up_b, K_W, K_b, V_W, V_b, d0_W, d0_b, d1_W, d1_b):
    """Pack all replicated constants into one (128, NC) f16 array + offset map."""
    P17 = positions.reshape(J, H).astype(np.float64)
    PA = (P17 @ up_W[DIM:].astype(np.float64) + up_b.astype(np.float64)).astype(np.float32)  # (17,32)
    Pq64 = P17 / np.sqrt(DIM)
    PqK = (Pq64 @ K_W.astype(np.float64).T).astype(np.float32)        # (17,32): gather commutes with K_W
    pqkb = (Pq64 @ K_b.astype(np.float64)).astype(np.float32)         # (17,)
    Wx0 = up_W[0].astype(np.float32)                                  # (32,)
    Wx1 = up_W[1].astype(np.float32)
    VW2 = (V_W.astype(np.float64) @ d0_W.astype(np.float64))
    Vb2 = (V_b.astype(np.float64) @ d0_W.astype(np.float64) + d0_b.astype(np.float64)).astype(np.float32)
    VW2T = np.ascontiguousarray(VW2.T).astype(np.float32)             # VW2T[h',h]
    d1WT = np.ascontiguousarray(d1_W.T).astype(np.float32)            # d1WT[h',h]
    Ltri = np.tril(np.ones((J, J), dtype=np.float32))                 # Ltri[j,j'] = 1 if j'<=j
    iota = np.arange(J, dtype=np.float32)
    c11 = 12.0 + iota                                                 # (12+j)
    c13 = 13.0 + iota

    parts = [
        ("VW2T", VW2T.reshape(-1)), ("d1WT", d1WT.reshape(-1)),
        ("PA", PA.reshape(-1)), ("PqK", PqK.reshape(-1)), ("pqkb", pqkb),
        ("Wx0", Wx0), ("Wx1", Wx1),
        ("Vb2", Vb2), ("d1b", d1_b.astype(np.float32)),
        ("Ltri", Ltri.reshape(-1)), ("iota", iota), ("c11", c11), ("c13", c13),
    ]
    offs = {}
    cur = 0
    vecs = []
    for name, v in parts:
        offs[name] = cur
        cur += v.size
        vecs.append(v.astype(np.float32))
    flat = np.concatenate(vecs)
    cst = np.tile(flat[None, :], (P, 1)).astype(np.float16)
    return cst, offs


def _build_bass(offs, NC, nt=NT):
    import concourse.bass as bass
    import concourse.mybir as mybir
    from concourse.tile import TileContext
    import concourse.tile_sem_assignment as _tsa
    _tsa.NUM_HWDGE_SEMS = 1  # all HWDGE DMAs on one sem lane: keeps tail drain <= 3 waits

    f32 = mybir.dt.float32
    f16 = mybir.dt.float16
    u8 = mybir.dt.uint8
    Alu = mybir.AluOpType
    Ax = mybir.AxisListType

    nc = bass.Bass()
    bs = nt * P
    NCB = NC + nt * 41
    bd = nc.dram_tensor("blob", [P, NCB], f16, kind="ExternalInput")
    oqd = nc.dram_tensor("out_q", [bs, N_MASK * H], u8, kind="ExternalOutput")
    osd = nc.dram_tensor("out_s", [P, nt], f16, kind="ExternalOutput")
    oqv = oqd[:, :].rearrange("(n p) c -> p n c", p=P)

    def bc(ap, shape):
        return ap.broadcast_to(shape)

    with nc.sbuf_tensor([P, NCB], f16) as blob16_t, \
         nc.sbuf_tensor([P, NCB], f32) as blob_t, \
         nc.sbuf_tensor([P, nt * 160], u8) as obuf_t, \
         nc.sbuf_tensor([P, nt], f16) as sbuf_s_t, \
         nc.semaphore() as psem, nc.semaphore() as osem:
        nc.sync.dma_start(out=blob16_t[:, :], in_=bd[:, :]).then_inc(psem, 16)
        nc.vector.wait_ge(psem, 16)
        blob = blob_t[:, :]
        obuf = obuf_t[:, :]
        sbuf_s = sbuf_s_t[:, :]
        with TileContext(nc) as tc, (
            tc.tile_pool(name="cpool", bufs=1)) as cpool, (
            tc.tile_pool(name="io", bufs=1)) as io, (
            tc.tile_pool(name="wk", bufs=1)) as wk, (
            tc.tile_pool(name="big", bufs=1)) as big:
            # widen the whole blob (consts + per-tile x/mask) to f32 once
            nc.vector.tensor_scalar_add(blob_t[:, :], blob16_t[:, :], 0.0)
            cst = blob[:, 0:NC]

            def C(name, n):
                o = offs[name]
                return cst[:, o:o + n]

            VW2T = C("VW2T", 1024).rearrange("p (g h) -> p g h", h=H)    # [h',h]
            d1WT = C("d1WT", 1024).rearrange("p (g h) -> p g h", h=H)
            PAc = C("PA", J * H)
            PqKc = C("PqK", J * H).rearrange("p (j h) -> p j h", h=H)
            pqkbc = C("pqkb", J)
            Wx0 = C("Wx0", H)
            Wx1 = C("Wx1", H)
            Vb2 = C("Vb2", H)
            d1b = C("d1b", H)
            Ltri = C("Ltri", J * J).rearrange("p (j k) -> p j k", k=J)
            iotaC = C("iota", J)
            c11 = C("c11", J)
            c13 = C("c13", J)

            for it in range(nt):
                base = NC + it * 41
                xt = blob[:, base:base + 24]
                mf = blob[:, base + 24:base + 41]

                # inclusive cumsum of mask: cv[b,j] = sum_{j'<=j} m[b,j']
                pr289 = wk.tile([P, J, J], f32, tag="pr289")
                nc.vector.tensor_tensor(pr289[:], Ltri,
                                        bc(mf.unsqueeze(1), (P, J, J)), Alu.mult)
                cv = wk.tile([P, J], f32, tag="cv")
                nc.vector.tensor_reduce(cv[:], pr289[:], axis=Ax.X, op=Alu.add)

                # perm = (m? cv-1 : 12+j-cv) = (c11 - cv) + m*(2cv - c13)
                t1 = wk.tile([P, J], f32, tag="t1")
                nc.vector.tensor_scalar_mul(t1[:], cv[:], 2.0)
                t2 = wk.tile([P, J], f32, tag="t2")
                nc.vector.tensor_tensor(t2[:], t1[:], c13, Alu.subtract)
                t3 = wk.tile([P, J], f32, tag="t3")
                nc.vector.tensor_tensor(t3[:], mf, t2[:], Alu.mult)
                t4 = wk.tile([P, J], f32, tag="t4")
                nc.vector.tensor_tensor(t4[:], c11, cv[:], Alu.subtract)
                perm = wk.tile([P, J], f32, tag="perm")
                nc.vector.tensor_tensor(perm[:], t4[:], t3[:], Alu.add)

                # one-hot G[b,j,s] = (perm[b,j] == s)
                G = wk.tile([P, J, J], f32, tag="G")
                nc.vector.tensor_tensor(
                    G[:], bc(perm[:, :].unsqueeze(2), (P, J, J)),
                    bc(iotaC.unsqueeze(1), (P, J, J)), Alu.is_equal)

                # xs[b,j,ch] = sum_r G[b,j,r] * x[b,r,ch]   (scatter x into 17 slots)
                pr408 = wk.tile([P, J, DIM, N_VIS], f32, tag="pr408")
                Gv = G[:, :, 0:N_VIS]  # (P,J,12)
                nc.vector.tensor_tensor(
                    pr408[:], bc(Gv.unsqueeze(2), (P, J, DIM, N_VIS)),
                    bc(xt.rearrange("p (r c) -> p r c", c=DIM)
                       .transpose([0, 2, 1]).unsqueeze(1), (P, J, DIM, N_VIS)),
                    Alu.mult)
                xs = wk.tile([P, J, DIM], f32, tag="xs")
                nc.vector.tensor_reduce(xs[:], pr408[:], axis=Ax.X, op=Alu.add)

                # qK[b,i,h] = sum_j G[b,j,12+i] * PqK[j,h]  (K_W pre-folded on host)
                pr2720 = big.tile([P, 5, H, J], f32, tag="big")
                Gm = G[:, :, N_VIS:J]  # (P,J,5)
                nc.vector.tensor_tensor(
                    pr2720[:],
                    bc(Gm.transpose([0, 2, 1]).unsqueeze(2), (P, 5, H, J)),
                    bc(PqKc.transpose([0, 2, 1]).unsqueeze(1), (P, 5, H, J)),
                    Alu.mult)
                qK = wk.tile([P, 5, H], f32, tag="qK")
                nc.vector.tensor_reduce(qK[:], pr2720[:], axis=Ax.X, op=Alu.add)

                # qKb[b,i] = sum_j G[b,j,12+i] * (Pq@K_b)[j]
                pr85 = wk.tile([P, 5, J], f32, tag="pr85")
                nc.vector.tensor_tensor(
                    pr85[:], Gm.transpose([0, 2, 1]),
                    bc(pqkbc.unsqueeze(1), (P, 5, J)), Alu.mult)
                qKb = wk.tile([P, 5], f32, tag="qKb")
                nc.vector.tensor_reduce(qKb[:], pr85[:], axis=Ax.X, op=Alu.add)

                # pre[b,j,h] = xs[b,j,0]*Wx0[h] + xs[b,j,1]*Wx1[h] + PA[j,h]
                tA = wk.tile([P, J, H], f32, tag="tA")
                nc.vector.tensor_tensor(
                    tA[:], bc(xs[:, :, 0:1], (P, J, H)),
                    bc(Wx0.unsqueeze(1), (P, J, H)), Alu.mult)
                tB = wk.tile([P, J, H], f32, tag="tB")
                nc.vector.tensor_tensor(
                    tB[:], bc(xs[:, :, 1:2], (P, J, H)),
                    bc(Wx1.unsqueeze(1), (P, J, H)), Alu.mult)
                pre = wk.tile([P, J, H], f32, tag="pre")
                nc.vector.tensor_tensor(pre[:], tA[:], tB[:], Alu.add)
                pre2 = wk.tile([P, J, H], f32, tag="pre2")
                nc.vector.tensor_tensor(
                    pre2[:], pre[:], PAc.rearrange("p (j h) -> p j h", h=H), Alu.add)

                # up = leaky_relu(pre2)
                tL = wk.tile([P, J, H], f32, tag="tL")
                nc.vector.tensor_scalar_mul(tL[:], pre2[:], 0.01)
                up = wk.tile([P, J, H], f32, tag="up")
                nc.vector.tensor_tensor(up[:], pre2[:], tL[:], Alu.max)

                # S[b,i,jk] = sum_h qK[b,i,h]*up[b,jk,h]  (+ qKb)
                prS = big.tile([P, 5, J, H], f32, tag="big")
                nc.vector.tensor_tensor(
                    prS[:], bc(qK[:].unsqueeze(2), (P, 5, J, H)),
                    bc(up[:].unsqueeze(1), (P, 5, J, H)), Alu.mult)
                S = wk.tile([P, 5, J], f32, tag="S")
                nc.vector.tensor_reduce(S[:], prS[:], axis=Ax.X, op=Alu.add)
                S2 = wk.tile([P, 5, J], f32, tag="S2")
                nc.vector.tensor_tensor(
                    S2[:], S[:], bc(qKb[:].unsqueeze(2), (P, 5, J)), Alu.add)

                # E = exp(S2) * m, exp via (poly(x/256))^256 -- DVE only
                zz = wk.tile([P, 5, J], f32, tag="zz")
                nc.vector.tensor_scalar_mul(zz[:], S2[:], 1.0 / 256.0)
                W1 = wk.tile([P, 5, J], f32, tag="W1")
                W2 = wk.tile([P, 5, J], f32, tag="W2")
                nc.vector.tensor_scalar(W1[:], zz[:], 1.0 / 24.0, 1.0 / 6.0,
                                        Alu.mult, Alu.add)
                for cconst in (0.5, 1.0, 1.0):
                    nc.vector.tensor_tensor(W2[:], W1[:], zz[:], Alu.mult)
                    nc.vector.tensor_scalar_add(W1[:], W2[:], cconst)
                for _sq in range(4):
                    nc.vector.tensor_tensor(W2[:], W1[:], W1[:], Alu.mult)
                    nc.vector.tensor_tensor(W1[:], W2[:], W2[:], Alu.mult)
                E2 = wk.tile([P, 5, J], f32, tag="E2")
                nc.vector.tensor_tensor(
                    E2[:], W1[:], bc(mf.unsqueeze(1), (P, 5, J)), Alu.mult)

                # Z, 1/Z
                Z = wk.tile([P, 5], f32, tag="Z")
                nc.vector.tensor_reduce(Z[:], E2[:], axis=Ax.X, op=Alu.add)
                rZ = wk.tile([P, 5], f32, tag="rZ")
                nc.vector.reciprocal(rZ[:], Z[:])

                # Eu[b,i,h] = sum_jk E2[b,i,jk]*up[b,jk,h]
                prE = big.tile([P, 5, H, J], f32, tag="big")
                nc.vector.tensor_tensor(
                    prE[:], bc(E2[:].unsqueeze(2), (P, 5, H, J)),
                    bc(up[:].transpose([0, 2, 1]).unsqueeze(1), (P, 5, H, J)),
                    Alu.mult)
                Eu = wk.tile([P, 5, H], f32, tag="Eu")
                nc.vector.tensor_reduce(Eu[:], prE[:], axis=Ax.X, op=Alu.add)

                # o1[b,i,h'] = sum_h Eu[b,i,h]*VW2[h,h']  (VW2T[h',h] layout)
                prO = big.tile([P, 5, H, H], f32, tag="big")
                nc.vector.tensor_tensor(
                    prO[:], bc(Eu[:].unsqueeze(2), (P, 5, H, H)),
                    bc(VW2T.unsqueeze(1), (P, 5, H, H)), Alu.mult)
                o1 = wk.tile([P, 5, H], f32, tag="o1")
                nc.vector.tensor_reduce(o1[:], prO[:], axis=Ax.X, op=Alu.add)

                # o1n = (o1 + Z*Vb2) / Z
                tZ = wk.tile([P, 5, H], f32, tag="tZ")
                nc.vector.tensor_tensor(
                    tZ[:], bc(Z[:].unsqueeze(2), (P, 5, H)),
                    bc(Vb2.unsqueeze(1), (P, 5, H)), Alu.mult)
                o1b = wk.tile([P, 5, H], f32, tag="o1b")
                nc.vector.tensor_tensor(o1b[:], o1[:], tZ[:], Alu.add)
                o1n = wk.tile([P, 5, H], f32, tag="o1n")
                nc.vector.tensor_tensor(
                    o1n[:], o1b[:], bc(rZ[:].unsqueeze(2), (P, 5, H)), Alu.mult)

                # lk = leaky(o1n)
                tL2 = wk.tile([P, 5, H], f32, tag="tL2")
                nc.vector.tensor_scalar_mul(tL2[:], o1n[:], 0.01)
                lk = wk.tile([P, 5, H], f32, tag="lk")
                nc.vector.tensor_tensor(lk[:], o1n[:], tL2[:], Alu.max)

                # out[b,i,h'] = sum_h lk[b,i,h]*d1_W[h,h'] + d1_b
                prD = big.tile([P, 5, H, H], f32, tag="big")
                nc.vector.tensor_tensor(
                    prD[:], bc(lk[:].unsqueeze(2), (P, 5, H, H)),
                    bc(d1WT.unsqueeze(1), (P, 5, H, H)), Alu.mult)
                ob = wk.tile([P, 5, H], f32, tag="ob")
                nc.vector.tensor_reduce(ob[:], prD[:], axis=Ax.X, op=Alu.add)
                obf = wk.tile([P, 5, H], f32, tag="obf")
                nc.vector.tensor_tensor(
                    obf[:], ob[:], bc(d1b.unsqueeze(1), (P, 5, H)), Alu.add)

                # int8 quantization with per-row scale
                of = obf[:].rearrange("p i h -> p (i h)")          # (P,160)
                rm = wk.tile([P, 1], f32, tag="rm")
                nc.vector.tensor_reduce(rm[:], of, axis=Ax.X, op=Alu.max,
                                        apply_absolute_value=True)
                rmc = wk.tile([P, 1], f32, tag="rmc")
                nc.vector.tensor_scalar_max(rmc[:], rm[:], 1e-12)
                rs = wk.tile([P, 1], f32, tag="rs")
                nc.vector.reciprocal(rs[:], rmc[:])
                ts = wk.tile([P, 1], f32, tag="ts")
                nc.vector.tensor_scalar_mul(ts[:], rs[:], QSCALE)
                # store per-row scale (f16): s = rm / QSCALE
                nc.vector.tensor_scalar_mul(
                    sbuf_s[:, it:it + 1], rmc[:], 1.0 / QSCALE)
                # q = out * (QSCALE/rm) + 128.0, converted to u8 on write
                qo = obuf[:, it * 160:(it + 1) * 160]
                nc.vector.tensor_scalar(qo, of, ts[:, :], 128.0,
                                        Alu.mult, Alu.add)
        nc.sync.dma_start(
            out=oqv, in_=obuf_t[:, :].rearrange("p (n c) -> p n c", c=160)
        ).then_inc(osem, 16)
        nc.sync.dma_start(
            out=osd[:, :], in_=sbuf_s_t[:, :]
        ).then_inc(osem, 16)
        nc.sync.wait_ge(osem, 32)

    return nc


def _install_fast_pjrt():
    """Memoize the jitted dispatch of run_bass_via_pjrt across calls.

    The stock implementation rebuilds the jit(shard_map(...)) closure on
    every call (full retrace + executable-cache lookup) and uploads freshly
    allocated zero buffers for the donated outputs over the axon tunnel.
    This cached version keeps the jitted callable alive and materializes the
    donation zeros on-device instead.
    """
    import jax
    import jax.numpy as jnp
    from jax.sharding import Mesh, PartitionSpec, NamedSharding
    from jax.experimental.shard_map import shard_map
    from concourse import bass2jax as b2j
    import concourse.mybir as mybir

    if getattr(b2j, "_fast_pjrt_patch", None) is not None:
        return
    cache = {}

    def fast_run_bass_via_pjrt(nc, in_maps, n_cores):
        key = (id(nc), n_cores)
        ent = cache.get(key)
        if ent is None:
            b2j.install_neuronx_cc_hook()
            if nc.dbg_addr is not None and nc.dbg_callbacks:
                raise RuntimeError("dbg_callbacks unsupported under axon")
            partition_name = (nc.partition_id_tensor.name
                              if nc.partition_id_tensor else None)
            param_names, out_names, out_avals, zero_shapes = [], [], [], []
            for alloc in nc.m.functions[0].allocations:
                if not isinstance(alloc, mybir.MemoryLocationSet):
                    continue
                name = alloc.memorylocations[0].name
                if alloc.kind == "ExternalInput":
                    if name != partition_name:
                        param_names.append(name)
                elif alloc.kind == "ExternalOutput":
                    shape = tuple(alloc.tensor_shape)
                    dtype = mybir.dt.np(alloc.dtype)
                    out_names.append(name)
                    out_avals.append(jax.core.ShapedArray(shape, dtype))
                    zero_shapes.append(((n_cores * shape[0], *shape[1:]), dtype))
            n_params = len(param_names)
            n_outs = len(out_avals)
            all_in = list(param_names) + list(out_names)
            if partition_name is not None:
                all_in.append(partition_name)
            donate = tuple(range(n_params, n_params + n_outs))

            def _body(*args):
                operands = list(args)
                if partition_name is not None:
                    operands.append(b2j.partition_id_tensor())
                outs = b2j._bass_exec_p.bind(
                    *operands, out_avals=tuple(out_avals),
                    in_names=tuple(all_in), out_names=tuple(out_names),
                    lowering_input_output_aliases=(),
                    sim_require_finite=True, sim_require_nnan=True, nc=nc)
                return tuple(outs)

            devices = jax.devices()[:n_cores]
            mesh = Mesh(np.asarray(devices), ("core",))
            in_specs = (PartitionSpec("core"),) * (n_params + n_outs)
            out_specs = (PartitionSpec("core"),) * n_outs
            sharded = jax.jit(
                shard_map(_body, mesh=mesh, in_specs=in_specs,
                          out_specs=out_specs, check_rep=False),
                donate_argnums=donate, keep_unused=True)
            zsh = NamedSharding(mesh, PartitionSpec("core"))

            def _zeros():
                return tuple(jnp.zeros(s, d) for s, d in zero_shapes)

            zfn = jax.jit(_zeros, out_shardings=(zsh,) * n_outs)
            ent = (param_names, out_names, out_avals, sharded, zfn,
                   nc.dbg_addr)
            cache[key] = ent
        param_names, out_names, out_avals, sharded, zfn, dbg_addr = ent
        zeros = zfn()  # async: device-side memset overlaps host-side concat
        if dbg_addr is not None:
            in_maps = [{**m, dbg_addr.name: np.zeros((1, 2), np.uint32)}
                       for m in in_maps]
        concat_in = [
            np.concatenate([np.asarray(in_maps[c][name])
                            for c in range(n_cores)], axis=0)
            for name in param_names]
        out_arrs = sharded(*concat_in, *zeros)
        return [
            {name: np.asarray(out_arrs[i]).reshape(
                n_cores, *out_avals[i].shape)[c]
             for i, name in enumerate(out_names)}
            for c in range(n_cores)]

    b2j._fast_pjrt_patch = b2j.run_bass_via_pjrt
    b2j.run_bass_via_pjrt = fast_run_bass_via_pjrt


_CACHE = {}


def kernel(x, m_bool, positions, up_W, up_b, K_W, K_b, V_W, V_b, d0_W, d0_b, d1_W, d1_b,
           _cache=_CACHE):
    import time as _time
    from concourse.bass_utils import run_bass_kernel_spmd
    _install_fast_pjrt()

    cst16, offs = _build_consts(positions, up_W, up_b, K_W, K_b, V_W, V_b,
                                d0_W, d0_b, d1_W, d1_b)
    NC = cst16.shape[1]
    if "nc" not in _cache:
        _cache["nc"] = _build_bass(offs, NC)
    nc = _cache["nc"]

    xm = np.empty((B, 41), np.float16)
    xm[:, :24] = x.reshape(B, 24)
    xm[:, 24:] = m_bool
    in_maps = []
    for c in range(NCORES):
        xmc = xm[c * BS:(c + 1) * BS].reshape(NT, P, 41).transpose(1, 0, 2).reshape(P, NT * 41)
        blob = np.concatenate([cst16, xmc], axis=1)
        in_maps.append({"blob": blob})

    _t0 = _time.time()
    res = run_bass_kernel_spmd(nc, in_maps, core_ids=list(range(NCORES)))
    _cache["exec_wall_ns"] = int((_time.time() - _t0) * 1e9)
    _cache["last_res"] = res

    q = np.concatenate([res.results[c]["out_q"] for c in range(NCORES)], axis=0)
    # out_s is [P, NT] (partition-major); batch row n*P+p -> s[p, n]
    s = np.concatenate(
        [res.results[c]["out_s"].T.reshape(BS, 1) for c in range(NCORES)], axis=0)
    # device u8 convert rounds to nearest on HW: q = round(out/s + 128)
    out = (q.astype(np.float32) - 128.0) * s.astype(np.float32)
    return out.reshape(B, N_MASK, H)


# revision 14
# speedup vs baseline: 1.5775x; 1.5775x over previous
import numpy as np

B, J, DIM, H = 131072, 17, 2, 32
N_VIS, N_MASK = 12, 5
NCORES = 8
BS = B // NCORES  # rows per core
P = 128           # rows per tile (partitions)
NT = BS // P      # tiles per core
QSCALE = 126.5    # int8 quantization range


def _build_consts(positions, up_W, up_b, K_W, K_b, V_W, V_b, d0_W, d0_b, d1_W, d1_b):
    """Pack all replicated constants into one (128, NC) f16 array + offset map."""
    P17 = positions.reshape(J, H).astype(np.float64)
    PA = (P17 @ up_W[DIM:].astype(np.float64) + up_b.astype(np.float64)).astype(np.float32)  # (17,32)
    Pq64 = P17 / np.sqrt(DIM)
    PqK = (Pq64 @ K_W.astype(np.float64).T).astype(np.float32)        # (17,32): gather commutes with K_W
    pqkb = (Pq64 @ K_b.astype(np.float64)).astype(np.float32)         # (17,)
    Wx0 = up_W[0].astype(np.float32)                                  # (32,)
    Wx1 = up_W[1].astype(np.float32)
    VW2 = (V_W.astype(np.float64) @ d0_W.astype(np.float64))
    Vb2 = (V_b.astype(np.float64) @ d0_W.astype(np.float64) + d0_b.astype(np.float64)).astype(np.float32)
    VW2T = np.ascontiguousarray(VW2.T).astype(np.float32)             # VW2T[h',h]
    d1WT = np.ascontiguousarray(d1_W.T).astype(np.float32)            # d1WT[h',h]
    Ltri = np.tril(np.ones((J, J), dtype=np.float32))                 # Ltri[j,j'] = 1 if j'<=j
    iota = np.arange(J, dtype=np.float32)
    c11 = 12.0 + iota                                                 # (12+j)
    c13 = 13.0 + iota

    parts = [
        ("VW2T", VW2T.reshape(-1)), ("d1WT", d1WT.reshape(-1)),
        ("PA", PA.reshape(-1)), ("PqK", PqK.reshape(-1)), ("pqkb", pqkb),
        ("Wx0", Wx0), ("Wx1", Wx1),
        ("Vb2", Vb2), ("d1b", d1_b.astype(np.float32)),
        ("Ltri", Ltri.reshape(-1)), ("iota", iota), ("c11", c11), ("c13", c13),
    ]
    offs = {}
    cur = 0
    vecs = []
    for name, v in parts:
        offs[name] = cur
        cur += v.size
        vecs.append(v.astype(np.float32))
    flat = np.concatenate(vecs)
    cst = flat[None, :].astype(np.float16)   # (1, NC): broadcast on device
    return cst, offs


def _build_bass(offs, NC, nt=NT):
    import concourse.bass as bass
    import concourse.mybir as mybir
    from concourse.tile import TileContext
    import concourse.tile_sem_assignment as _tsa
    _tsa.NUM_HWDGE_SEMS = 1  # all HWDGE DMAs on one sem lane: keeps tail drain <= 3 waits

    f32 = mybir.dt.float32
    f16 = mybir.dt.float16
    u8 = mybir.dt.uint8
    Alu = mybir.AluOpType
    Ax = mybir.AxisListType

    nc = bass.Bass()
    bs = nt * P
    OW = N_MASK * H + 2   # 160 u8 quantized values + f16 scale as 2 bytes
    bd = nc.dram_tensor("blob", [P, nt * 41], f16, kind="ExternalInput")
    cd = nc.dram_tensor("cst", [1, NC], f16, kind="ExternalInput")
    oqd = nc.dram_tensor("out_q", [bs, OW], u8, kind="ExternalOutput")
    oqv = oqd[:, :].rearrange("(n p) c -> p n c", p=P)

    def bc(ap, shape):
        return ap.broadcast_to(shape)

    with nc.sbuf_tensor([P, nt * 41], f16) as blob16_t, \
         nc.sbuf_tensor([P, NC], f16) as cst16_t, \
         nc.sbuf_tensor([P, nt * 41], f32) as blob_t, \
         nc.sbuf_tensor([P, NC], f32) as cst_t, \
         nc.sbuf_tensor([P, nt * OW], u8) as obuf_t, \
         nc.semaphore() as psem, nc.semaphore() as osem:
        nc.sync.dma_start(out=blob16_t[:, :], in_=bd[:, :]).then_inc(psem, 16)
        nc.sync.dma_start(
            out=cst16_t[:, :], in_=cd[:, :].broadcast_to((P, NC))
        ).then_inc(psem, 16)
        nc.vector.wait_ge(psem, 32)
        blob = blob_t[:, :]
        obuf = obuf_t[:, :]
        with TileContext(nc) as tc, (
            tc.tile_pool(name="cpool", bufs=1)) as cpool, (
            tc.tile_pool(name="io", bufs=1)) as io, (
            tc.tile_pool(name="wk", bufs=1)) as wk, (
            tc.tile_pool(name="big", bufs=1)) as big:
            # widen per-tile data and consts to f32 once
            nc.vector.tensor_scalar_add(blob_t[:, :], blob16_t[:, :], 0.0)
            nc.vector.tensor_scalar_add(cst_t[:, :], cst16_t[:, :], 0.0)
            cst = cst_t[:, :]

            def C(name, n):
                o = offs[name]
                return cst[:, o:o + n]

            VW2T = C("VW2T", 1024).rearrange("p (g h) -> p g h", h=H)    # [h',h]
            d1WT = C("d1WT", 1024).rearrange("p (g h) -> p g h", h=H)
            PAc = C("PA", J * H)
            PqKc = C("PqK", J * H).rearrange("p (j h) -> p j h", h=H)
            pqkbc = C("pqkb", J)
            Wx0 = C("Wx0", H)
            Wx1 = C("Wx1", H)
            Vb2 = C("Vb2", H)
            d1b = C("d1b", H)
            Ltri = C("Ltri", J * J).rearrange("p (j k) -> p j k", k=J)
            iotaC = C("iota", J)
            c11 = C("c11", J)
            c13 = C("c13", J)

            for it in range(nt):
                base = it * 41
                xt = blob[:, base:base + 24]
                mf = blob[:, base + 24:base + 41]

                # inclusive cumsum of mask: cv[b,j] = sum_{j'<=j} m[b,j']
                pr289 = wk.tile([P, J, J], f32, tag="pr289")
                nc.vector.tensor_tensor(pr289[:], Ltri,
                                        bc(mf.unsqueeze(1), (P, J, J)), Alu.mult)
                cv = wk.tile([P, J], f32, tag="cv")
                nc.vector.tensor_reduce(cv[:], pr289[:], axis=Ax.X, op=Alu.add)

                # perm = (m? cv-1 : 12+j-cv) = (c11 - cv) + m*(2cv - c13)
                t1 = wk.tile([P, J], f32, tag="t1")
                nc.vector.tensor_scalar_mul(t1[:], cv[:], 2.0)
                t2 = wk.tile([P, J], f32, tag="t2")
                nc.vector.tensor_tensor(t2[:], t1[:], c13, Alu.subtract)
                t3 = wk.tile([P, J], f32, tag="t3")
                nc.vector.tensor_tensor(t3[:], mf, t2[:], Alu.mult)
                t4 = wk.tile([P, J], f32, tag="t4")
                nc.vector.tensor_tensor(t4[:], c11, cv[:], Alu.subtract)
                perm = wk.tile([P, J], f32, tag="perm")
                nc.vector.tensor_tensor(perm[:], t4[:], t3[:], Alu.add)

                # one-hot G[b,j,s] = (perm[b,j] == s)
                G = wk.tile([P, J, J], f32, tag="G")
                nc.vector.tensor_tensor(
                    G[:], bc(perm[:, :].unsqueeze(2), (P, J, J)),
                    bc(iotaC.unsqueeze(1), (P, J, J)), Alu.is_equal)

                # xs[b,j,ch] = sum_r G[b,j,r] * x[b,r,ch]   (scatter x into 17 slots)
                pr408 = wk.tile([P, J, DIM, N_VIS], f32, tag="pr408")
                Gv = G[:, :, 0:N_VIS]  # (P,J,12)
                nc.vector.tensor_tensor(
                    pr408[:], bc(Gv.unsqueeze(2), (P, J, DIM, N_VIS)),
                    bc(xt.rearrange("p (r c) -> p r c", c=DIM)
                       .transpose([0, 2, 1]).unsqueeze(1), (P, J, DIM, N_VIS)),
                    Alu.mult)
                xs = wk.tile([P, J, DIM], f32, tag="xs")
                nc.vector.tensor_reduce(xs[:], pr408[:], axis=Ax.X, op=Alu.add)

                # qK[b,i,h] = sum_j G[b,j,12+i] * PqK[j,h]  (K_W pre-folded on host)
                pr2720 = big.tile([P, 5, H, J], f32, tag="big")
                Gm = G[:, :, N_VIS:J]  # (P,J,5)
                nc.vector.tensor_tensor(
                    pr2720[:],
                    bc(Gm.transpose([0, 2, 1]).unsqueeze(2), (P, 5, H, J)),
                    bc(PqKc.transpose([0, 2, 1]).unsqueeze(1), (P, 5, H, J)),
                    Alu.mult)
                qK = wk.tile([P, 5, H], f32, tag="qK")
                nc.vector.tensor_reduce(qK[:], pr2720[:], axis=Ax.X, op=Alu.add)

                # qKb[b,i] = sum_j G[b,j,12+i] * (Pq@K_b)[j]
                pr85 = wk.tile([P, 5, J], f32, tag="pr85")
                nc.vector.tensor_tensor(
                    pr85[:], Gm.transpose([0, 2, 1]),
                    bc(pqkbc.unsqueeze(1), (P, 5, J)), Alu.mult)
                qKb = wk.tile([P, 5], f32, tag="qKb")
                nc.vector.tensor_reduce(qKb[:], pr85[:], axis=Ax.X, op=Alu.add)

                # pre[b,j,h] = xs[b,j,0]*Wx0[h] + xs[b,j,1]*Wx1[h] + PA[j,h]
                tA = wk.tile([P, J, H], f32, tag="tA")
                nc.vector.tensor_tensor(
                    tA[:], bc(xs[:, :, 0:1], (P, J, H)),
                    bc(Wx0.unsqueeze(1), (P, J, H)), Alu.mult)
                tB = wk.tile([P, J, H], f32, tag="tB")
                nc.vector.tensor_tensor(
                    tB[:], bc(xs[:, :, 1:2], (P, J, H)),
                    bc(Wx1.unsqueeze(1), (P, J, H)), Alu.mult)
                pre = wk.tile([P, J, H], f32, tag="pre")
                nc.vector.tensor_tensor(pre[:], tA[:], tB[:], Alu.add)
                pre2 = wk.tile([P, J, H], f32, tag="pre2")
                nc.vector.tensor_tensor(
                    pre2[:], pre[:], PAc.rearrange("p (j h) -> p j h", h=H), Alu.add)

                # up = leaky_relu(pre2)
                tL = wk.tile([P, J, H], f32, tag="tL")
                nc.vector.tensor_scalar_mul(tL[:], pre2[:], 0.01)
                up = wk.tile([P, J, H], f32, tag="up")
                nc.vector.tensor_tensor(up[:], pre2[:], tL[:], Alu.max)

                # S[b,i,jk] = sum_h qK[b,i,h]*up[b,jk,h]  (+ qKb)
                prS = big.tile([P, 5, J, H], f32, tag="big")
                nc.vector.tensor_tensor(
                    prS[:], bc(qK[:].unsqueeze(2), (P, 5, J, H)),
                    bc(up[:].unsqueeze(1), (P, 5, J, H)), Alu.mult)
                S = wk.tile([P, 5, J], f32, tag="S")
                nc.vector.tensor_reduce(S[:], prS[:], axis=Ax.X, op=Alu.add)
                S2 = wk.tile([P, 5, J], f32, tag="S2")
                nc.vector.tensor_tensor(
                    S2[:], S[:], bc(qKb[:].unsqueeze(2), (P, 5, J)), Alu.add)

                # E = exp(S2) * m, exp via (poly(x/256))^256 -- DVE only
                zz = wk.tile([P, 5, J], f32, tag="zz")
                nc.vector.tensor_scalar_mul(zz[:], S2[:], 1.0 / 256.0)
                W1 = wk.tile([P, 5, J], f32, tag="W1")
                W2 = wk.tile([P, 5, J], f32, tag="W2")
                nc.vector.tensor_scalar(W1[:], zz[:], 1.0 / 24.0, 1.0 / 6.0,
                                        Alu.mult, Alu.add)
                for cconst in (0.5, 1.0, 1.0):
                    nc.vector.tensor_tensor(W2[:], W1[:], zz[:], Alu.mult)
                    nc.vector.tensor_scalar_add(W1[:], W2[:], cconst)
                for _sq in range(4):
                    nc.vector.tensor_tensor(W2[:], W1[:], W1[:], Alu.mult)
                    nc.vector.tensor_tensor(W1[:], W2[:], W2[:], Alu.mult)
                E2 = wk.tile([P, 5, J], f32, tag="E2")
                nc.vector.tensor_tensor(
                    E2[:], W1[:], bc(mf.unsqueeze(1), (P, 5, J)), Alu.mult)

                # Z, 1/Z
                Z = wk.tile([P, 5], f32, tag="Z")
                nc.vector.tensor_reduce(Z[:], E2[:], axis=Ax.X, op=Alu.add)
                rZ = wk.tile([P, 5], f32, tag="rZ")
                nc.vector.reciprocal(rZ[:], Z[:])

                # Eu[b,i,h] = sum_jk E2[b,i,jk]*up[b,jk,h]
                prE = big.tile([P, 5, H, J], f32, tag="big")
                nc.vector.tensor_tensor(
                    prE[:], bc(E2[:].unsqueeze(2), (P, 5, H, J)),
                    bc(up[:].transpose([0, 2, 1]).unsqueeze(1), (P, 5, H, J)),
                    Alu.mult)
                Eu = wk.tile([P, 5, H], f32, tag="Eu")
                nc.vector.tensor_reduce(Eu[:], prE[:], axis=Ax.X, op=Alu.add)

                # o1[b,i,h'] = sum_h Eu[b,i,h]*VW2[h,h']  (VW2T[h',h] layout)
                prO = big.tile([P, 5, H, H], f32, tag="big")
                nc.vector.tensor_tensor(
                    prO[:], bc(Eu[:].unsqueeze(2), (P, 5, H, H)),
                    bc(VW2T.unsqueeze(1), (P, 5, H, H)), Alu.mult)
                o1 = wk.tile([P, 5, H], f32, tag="o1")
                nc.vector.tensor_reduce(o1[:], prO[:], axis=Ax.X, op=Alu.add)

                # o1n = (o1 + Z*Vb2) / Z
                tZ = wk.tile([P, 5, H], f32, tag="tZ")
                nc.vector.tensor_tensor(
                    tZ[:], bc(Z[:].unsqueeze(2), (P, 5, H)),
                    bc(Vb2.unsqueeze(1), (P, 5, H)), Alu.mult)
                o1b = wk.tile([P, 5, H], f32, tag="o1b")
                nc.vector.tensor_tensor(o1b[:], o1[:], tZ[:], Alu.add)
                o1n = wk.tile([P, 5, H], f32, tag="o1n")
                nc.vector.tensor_tensor(
                    o1n[:], o1b[:], bc(rZ[:].unsqueeze(2), (P, 5, H)), Alu.mult)

                # lk = leaky(o1n)
                tL2 = wk.tile([P, 5, H], f32, tag="tL2")
                nc.vector.tensor_scalar_mul(tL2[:], o1n[:], 0.01)
                lk = wk.tile([P, 5, H], f32, tag="lk")
                nc.vector.tensor_tensor(lk[:], o1n[:], tL2[:], Alu.max)

                # out[b,i,h'] = sum_h lk[b,i,h]*d1_W[h,h'] + d1_b
                prD = big.tile([P, 5, H, H], f32, tag="big")
                nc.vector.tensor_tensor(
                    prD[:], bc(lk[:].unsqueeze(2), (P, 5, H, H)),
                    bc(d1WT.unsqueeze(1), (P, 5, H, H)), Alu.mult)
                ob = wk.tile([P, 5, H], f32, tag="ob")
                nc.vector.tensor_reduce(ob[:], prD[:], axis=Ax.X, op=Alu.add)
                obf = wk.tile([P, 5, H], f32, tag="obf")
                nc.vector.tensor_tensor(
                    obf[:], ob[:], bc(d1b.unsqueeze(1), (P, 5, H)), Alu.add)

                # int8 quantization with per-row scale
                of = obf[:].rearrange("p i h -> p (i h)")          # (P,160)
                rm = wk.tile([P, 1], f32, tag="rm")
                nc.vector.tensor_reduce(rm[:], of, axis=Ax.X, op=Alu.max,
                                        apply_absolute_value=True)
                rmc = wk.tile([P, 1], f32, tag="rmc")
                nc.vector.tensor_scalar_max(rmc[:], rm[:], 1e-12)
                rs = wk.tile([P, 1], f32, tag="rs")
                nc.vector.reciprocal(rs[:], rmc[:])
                ts = wk.tile([P, 1], f32, tag="ts")
                nc.vector.tensor_scalar_mul(ts[:], rs[:], QSCALE)
                # per-row scale s = rm / QSCALE, stored f16 in bytes 160:162
                sc_view = obuf[:, it * OW + 160:it * OW + 162].bitcast(f16)
                nc.vector.tensor_scalar_mul(sc_view, rmc[:], 1.0 / QSCALE)
                # q = out * (QSCALE/rm) + 128.0, converted to u8 on write
                qo = obuf[:, it * OW:it * OW + 160]
                nc.vector.tensor_scalar(qo, of, ts[:, :], 128.0,
                                        Alu.mult, Alu.add)
        nc.sync.dma_start(
            out=oqv, in_=obuf_t[:, :].rearrange("p (n c) -> p n c", c=OW)
        ).then_inc(osem, 16)
        nc.sync.wait_ge(osem, 16)

    return nc


def _install_fast_pjrt():
    """Memoize the jitted dispatch of run_bass_via_pjrt across calls.

    The stock implementation rebuilds the jit(shard_map(...)) closure on
    every call (full retrace + executable-cache lookup) and uploads freshly
    allocated zero buffers for the donated outputs over the axon tunnel.
    This cached version keeps the jitted callable alive and materializes the
    donation zeros on-device instead.
    """
    import jax
    import jax.numpy as jnp
    from jax.sharding import Mesh, PartitionSpec, NamedSharding
    from jax.experimental.shard_map import shard_map
    from concourse import bass2jax as b2j
    import concourse.mybir as mybir

    if getattr(b2j, "_fast_pjrt_patch", None) is not None:
        return
    cache = {}

    def fast_run_bass_via_pjrt(nc, in_maps, n_cores):
        key = (id(nc), n_cores)
        ent = cache.get(key)
        if ent is None:
            b2j.install_neuronx_cc_hook()
            if nc.dbg_addr is not None and nc.dbg_callbacks:
                raise RuntimeError("dbg_callbacks unsupported under axon")
            partition_name = (nc.partition_id_tensor.name
                              if nc.partition_id_tensor else None)
            param_names, out_names, out_avals, zero_shapes = [], [], [], []
            for alloc in nc.m.functions[0].allocations:
                if not isinstance(alloc, mybir.MemoryLocationSet):
                    continue
                name = alloc.memorylocations[0].name
                if alloc.kind == "ExternalInput":
                    if name != partition_name:
                        param_names.append(name)
                elif alloc.kind == "ExternalOutput":
                    shape = tuple(alloc.tensor_shape)
                    dtype = mybir.dt.np(alloc.dtype)
                    out_names.append(name)
                    out_avals.append(jax.core.ShapedArray(shape, dtype))
                    zero_shapes.append(((n_cores * shape[0], *shape[1:]), dtype))
            n_params = len(param_names)
            n_outs = len(out_avals)
            all_in = list(param_names) + list(out_names)
            if partition_name is not None:
                all_in.append(partition_name)
            donate = tuple(range(n_params, n_params + n_outs))

            def _body(*args):
                operands = list(args)
                if partition_name is not None:
                    operands.append(b2j.partition_id_tensor())
                outs = b2j._bass_exec_p.bind(
                    *operands, out_avals=tuple(out_avals),
                    in_names=tuple(all_in), out_names=tuple(out_names),
                    lowering_input_output_aliases=(),
                    sim_require_finite=True, sim_require_nnan=True, nc=nc)
                return tuple(outs)

            devices = jax.devices()[:n_cores]
            mesh = Mesh(np.asarray(devices), ("core",))
            in_specs = (PartitionSpec("core"),) * (n_params + n_outs)
            out_specs = (PartitionSpec("core"),) * n_outs
            sharded = jax.jit(
                shard_map(_body, mesh=mesh, in_specs=in_specs,
                          out_specs=out_specs, check_rep=False),
                donate_argnums=donate, keep_unused=True)
            zsh = NamedSharding(mesh, PartitionSpec("core"))

            def _zeros():
                return tuple(jnp.zeros(s, d) for s, d in zero_shapes)

            zfn = jax.jit(_zeros, out_shardings=(zsh,) * n_outs)
            ent = (param_names, out_names, out_avals, sharded, zfn,
                   nc.dbg_addr)
            cache[key] = ent
        param_names, out_names, out_avals, sharded, zfn, dbg_addr = ent
        zeros = zfn()  # async: device-side memset overlaps host-side concat
        if dbg_addr is not None:
            in_maps = [{**m, dbg_addr.name: np.zeros((1, 2), np.uint32)}
                       for m in in_maps]
        concat_in = [
            np.concatenate([np.asarray(in_maps[c][name])
                            for c in range(n_cores)], axis=0)
            for name in param_names]
        out_arrs = sharded(*concat_in, *zeros)
        # fetch per-device shards in parallel (shard c == core c's output)
        from concurrent.futures import ThreadPoolExecutor
        fetched = {}
        with ThreadPoolExecutor(8 * len(out_names)) as ex:
            futs = {}
            for i, name in enumerate(out_names):
                shards = sorted(out_arrs[i].addressable_shards,
                                key=lambda sh: sh.index[0].start or 0)
                futs[name] = [ex.submit(lambda d=sh.data: np.asarray(d))
                              for sh in shards]
            for name, fl in futs.items():
                fetched[name] = [f.result() for f in fl]
        return [{name: fetched[name][c] for name in out_names}
                for c in range(n_cores)]

    b2j._fast_pjrt_patch = b2j.run_bass_via_pjrt
    b2j.run_bass_via_pjrt = fast_run_bass_via_pjrt


_CACHE = {}


def kernel(x, m_bool, positions, up_W, up_b, K_W, K_b, V_W, V_b, d0_W, d0_b, d1_W, d1_b,
           _cache=_CACHE):
    import time as _time
    from concourse.bass_utils import run_bass_kernel_spmd
    _install_fast_pjrt()

    cst16, offs = _build_consts(positions, up_W, up_b, K_W, K_b, V_W, V_b,
                                d0_W, d0_b, d1_W, d1_b)
    NC = cst16.shape[1]
    if "nc" not in _cache:
        _cache["nc"] = _build_bass(offs, NC)
    nc = _cache["nc"]

    xm = np.empty((B, 41), np.float16)
    xm[:, :24] = x.reshape(B, 24)
    xm[:, 24:] = m_bool
    in_maps = []
    for c in range(NCORES):
        xmc = np.ascontiguousarray(
            xm[c * BS:(c + 1) * BS].reshape(NT, P, 41).transpose(1, 0, 2)
        ).reshape(P, NT * 41)
        in_maps.append({"blob": xmc, "cst": cst16})

    _t0 = _time.time()
    res = run_bass_kernel_spmd(nc, in_maps, core_ids=list(range(NCORES)))
    _cache["exec_wall_ns"] = int((_time.time() - _t0) * 1e9)
    _cache["last_res"] = res

    rows = np.concatenate([res.results[c]["out_q"] for c in range(NCORES)], axis=0)
    q = rows[:, :160]
    s = np.ascontiguousarray(rows[:, 160:162]).view(np.float16)
    # device u8 convert rounds to nearest on HW: q = round(out/s + 128)
    out = (q.astype(np.float32) - 128.0) * s.astype(np.float32)
    return out.reshape(B, N_MASK, H)
